# revision 9
# baseline (speedup 1.0000x reference)
"""COIL sparse-attention scoring kernel for 8 Trainium2 NeuronCores.

Strategy
--------
Shard the doc axis (Bd=128) across the 8 cores (16 docs each); qry tensors are
replicated. The exact-token-match mask is folded INTO the matmul: each token id
(vocab 1000) is encoded as three base-10 digit one-hots scaled by ALPHA=32 and
appended to the reps. Then

    v[qs, ct] = <qry_ext[qs], doc_ext[ct]> = S[qs, ct] + 1024 * match_digits

where match_digits == 3 iff the ids are equal, so

    tok[qs, c] = relu(max_t v[qs, c, t] - 3072)

reproduces the reference masked-max exactly (non-match scores sit below 2100,
matches above 3000). The qry reps are split hi/lo in bf16 (3 cross terms) so
the matmul runs at full bf16 rate with ~fp32 accuracy: K = 3*32 + 30 = 126.

Per core: 32 q-tiles of 128 q-positions; each q-tile is one [126,128]x[126,2048]
matmul into PSUM [128, 2048]. The per-doc max over the 128 doc tokens is split
between the DVE (direct tensor_reduce from PSUM) and a ScalarE relu-convert to
fp16 followed by a DVE tensor_tensor max tree at 2x rate. The sum over query
positions is a ones-vector matmul on the PE (partition-dim sum). CLS scores,
the tiny 4-way q-tile fold, and the final max over the 8 query chunks are done
on host (a few thousand elements).
"""

import os
import numpy as np
import ml_dtypes

Bq, Sq, Bd, Sd, D, Dc = 8, 512, 128, 128, 32, 768
NCORES = 8
BD_PER = Bd // NCORES          # 16 docs per core
K_EXT = 126                    # 32*3 rep dims + 30 one-hot dims
SQF = Bq * Sq                  # 4096 query positions
DCOL = BD_PER * Sd             # 2048 doc tokens per core
NQT = SQF // 128               # 32 q-tiles
ALPHA = 32.0
OFF = 3.0 * ALPHA * ALPHA      # 3072: offset of a full 3-digit match
D_DIRECT = int(os.environ.get("KERNEL_D_DIRECT", "3"))  # docs/qtile on DVE-direct path
TREE_LEVELS = int(os.environ.get("KERNEL_TREE_LEVELS", "3"))

_CACHE = {}


def _bf16(x):
    return x.astype(ml_dtypes.bfloat16)


def _onehot_digits(ids):
    """ids [N] int in [0,1000) -> [N,30] base-10 digit one-hot (float32)."""
    n = ids.shape[0]
    H = np.zeros((n, 30), dtype=np.float32)
    r = np.arange(n)
    H[r, ids % 10] = 1.0
    H[r, 10 + (ids // 10) % 10] = 1.0
    H[r, 20 + ids // 100] = 1.0
    return H


def _build_qry_ext(qry_reps, qry_input_ids, qry_attention_mask):
    q = np.asarray(qry_reps, np.float32).reshape(SQF, D)
    ids = np.asarray(qry_input_ids, np.int64).reshape(SQF)
    q_hi = _bf16(q).astype(np.float32)
    q_lo = _bf16(q - q_hi).astype(np.float32)
    H = ALPHA * _onehot_digits(ids)
    ext = np.concatenate([q_hi, q_lo, q_hi, H], axis=1)  # [SQF, 126]
    # rows that must contribute 0: CLS (s=0), SEP (last attended pos), mask==0
    mask = np.asarray(qry_attention_mask, np.int64).copy()
    sep = mask.sum(axis=1) - 1
    mask[np.arange(Bq), sep] = 0
    mask[:, 0] = 0
    ext *= mask.reshape(SQF, 1).astype(np.float32)
    return np.ascontiguousarray(_bf16(ext).T)  # [126, SQF]


def _build_doc_ext(doc_reps, doc_input_ids):
    d = np.asarray(doc_reps, np.float32).reshape(-1, D)
    ids = np.asarray(doc_input_ids, np.int64).reshape(-1)
    d_hi = _bf16(d).astype(np.float32)
    d_lo = _bf16(d - d_hi).astype(np.float32)
    H = ALPHA * _onehot_digits(ids)
    ext = np.concatenate([d_hi, d_hi, d_lo, H], axis=1)  # [N, 126]
    return np.ascontiguousarray(_bf16(ext).T)  # [126, N]


def _split_multi_waits(nc, mybir):
    """This container's walrus accepts only ONE sync-wait per instruction
    ("Too many sync wait commands"). Hoist extra waits into standalone
    EventSemaphore instructions on the same engine right before the offender
    (the sequencer blocks on each in order — semantically identical)."""
    n = 0
    for func in nc.m.functions:
        for bb in func.blocks:
            out = []
            for inst in bb.instructions:
                si = inst.sync_info
                if si is not None and len(si.on_wait) > 1:
                    waits = list(si.on_wait)
                    for w in waits[:-1]:
                        n += 1
                        out.append(
                            mybir.InstEventSemaphore(
                                name=f"W-{inst.name}-{n}",
                                engine=inst.engine,
                                ins=[],
                                outs=[],
                                debug=inst.debug,
                                sync_info=mybir.SyncInfo(
                                    on_wait=[w], on_update=[]
                                ),
                            )
                        )
                    inst.sync_info = mybir.SyncInfo(
                        on_wait=[waits[-1]], on_update=list(si.on_update)
                    )
                out.append(inst)
            bb.instructions = out
    return n


def _build_nc(d_direct, tree_levels):
    import concourse.bass as bass
    import concourse.mybir as mybir
    import concourse.tile as tile
    from concourse.bass import ts

    bf16, f16, f32 = mybir.dt.bfloat16, mybir.dt.float16, mybir.dt.float32
    nc = bass.Bass("TRN2", target_bir_lowering=False, debug=False)
    qryT = nc.dram_tensor("qryT", [K_EXT, SQF], bf16, kind="ExternalInput").ap()
    docT = nc.dram_tensor("docT", [K_EXT, DCOL], bf16, kind="ExternalInput").ap()
    out = nc.dram_tensor("out", [1, 16 * NQT], f32, kind="ExternalOutput").ap()

    nd = BD_PER - d_direct
    with tile.TileContext(nc) as tc:
        with (
            tc.tile_pool(name="inp", bufs=1) as inp,
            tc.tile_pool(name="psum", bufs=2, space="PSUM") as psum,
            tc.tile_pool(name="stage", bufs=3) as stp,
            tc.tile_pool(name="tree", bufs=2) as trp,
            tc.tile_pool(name="accp", bufs=1) as accp,
        ):
            qry_sb = inp.tile([K_EXT, SQF], bf16)
            for j in range(SQF // 512):
                nc.sync.dma_start(qry_sb[:, ts(j, 512)], qryT[:, ts(j, 512)])
            doc_sb = inp.tile([K_EXT, DCOL], bf16)
            for j in range(DCOL // 512):
                nc.sync.dma_start(doc_sb[:, ts(j, 512)], docT[:, ts(j, 512)])

            accum = accp.tile([128, 16 * NQT], f32)
            draw = accp.tile([128, max(d_direct, 1) * NQT], f32)
            ones = accp.tile([128, 1], f32)
            nc.vector.memset(ones[:], 1.0)

            for qt in range(NQT):
                ps = psum.tile([128, DCOL], f32, tag="score")
                for j in range(DCOL // 512):
                    nc.tensor.matmul(
                        ps[:, ts(j, 512)],
                        qry_sb[:, ts(qt, 128)],
                        doc_sb[:, ts(j, 512)],
                        start=True,
                        stop=True,
                    )
                if d_direct:
                    nc.vector.reduce_max(
                        draw[:, qt * d_direct : (qt + 1) * d_direct],
                        ps[:, 0 : d_direct * Sd].rearrange("p (c t) -> p c t", t=Sd),
                        axis=mybir.AxisListType.X,
                    )
                if nd:
                    # fp16 copy of v - OFF; the relu is applied after the max
                    # tree (max commutes with both the shift and the relu)
                    st = stp.tile([128, nd * Sd], f16, tag="stage")
                    nc.scalar.activation(
                        st[:],
                        ps[:, d_direct * Sd : DCOL],
                        mybir.ActivationFunctionType.Copy,
                        bias=-OFF,
                    )
                    cur, width = st, Sd
                    for lev in range(tree_levels):
                        nxt = trp.tile([128, nd * width // 2], f16, tag=f"t{lev}")
                        cv = cur[:].rearrange("p (c t) -> p c t", t=width)
                        nc.vector.tensor_max(
                            nxt[:].rearrange("p (c t) -> p c t", t=width // 2),
                            cv[:, :, 0 : width // 2],
                            cv[:, :, width // 2 : width],
                        )
                        cur, width = nxt, width // 2
                    nc.vector.reduce_max(
                        accum[:, qt * 16 + d_direct : (qt + 1) * 16],
                        cur[:].rearrange("p (c t) -> p c t", t=width),
                        axis=mybir.AxisListType.X,
                    )
            if d_direct:
                # tok = max(raw, OFF) - OFF, scattered into the accum layout
                ov = accum[:].rearrange("p (q c) -> p q c", c=16)[:, :, 0:d_direct]
                iv = draw[:].rearrange("p (q c) -> p q c", c=d_direct)
                nc.vector.tensor_scalar(
                    ov, iv, OFF, -OFF, mybir.AluOpType.max, mybir.AluOpType.add
                )
            if nd:
                # tree cols hold max_t(v) - OFF; apply the relu in place
                tv = accum[:].rearrange("p (q c) -> p q c", c=16)[:, :, d_direct:16]
                nc.vector.tensor_scalar_max(tv, tv, 0.0)
            # partition-dim sum over the 128 q-positions of each q-tile
            fin = psum.tile([1, 16 * NQT], f32, tag="score")
            nc.tensor.matmul(fin[:], ones[:], accum[:], start=True, stop=True)
            osb = accp.tile([1, 16 * NQT], f32)
            nc.vector.tensor_copy(osb[:], fin[:])
            nc.sync.dma_start(out[:], osb[:])
    _split_multi_waits(nc, mybir)
    return nc


def _get_nc():
    key = (D_DIRECT, TREE_LEVELS)
    if key not in _CACHE:
        _CACHE[key] = _build_nc(*key)
    return _CACHE[key]


def _prepare_in_maps(inputs):
    qT = _build_qry_ext(
        inputs["qry_reps"], inputs["qry_input_ids"], inputs["qry_attention_mask"]
    )
    doc_reps = np.asarray(inputs["doc_reps"], np.float32)
    doc_ids = np.asarray(inputs["doc_input_ids"], np.int64)
    in_maps = []
    for core in range(NCORES):
        sl = slice(core * BD_PER, (core + 1) * BD_PER)
        dT = _build_doc_ext(doc_reps[sl], doc_ids[sl])
        in_maps.append({"qryT": qT, "docT": dT})
    return in_maps


def _assemble(inputs, results):
    toks = np.zeros((Bq, Bd), dtype=np.float32)
    for core in range(NCORES):
        part = np.asarray(results[core]["out"], np.float32).reshape(NQT, 16)
        # col-block layout: [q-tile, doc]; 4 q-tiles per query chunk
        toks[:, core * BD_PER : (core + 1) * BD_PER] = part.reshape(Bq, 4, 16).sum(1)
    cls = np.asarray(inputs["qry_cls"], np.float32) @ np.asarray(
        inputs["doc_cls"], np.float32
    ).T
    scores = toks + cls
    return scores.max(axis=0).reshape(-1).astype(np.float32)


def _ensure_ntff_hook():
    """This container's antenv lacks axon_hooks; synthesize the module and
    register the ctypes-based NTFF profile hook so trace=True works."""
    import sys
    import types

    if "antenv.axon_hooks" in sys.modules:
        return
    mod = types.ModuleType("antenv.axon_hooks")
    state = {"hook": None}
    mod.set_axon_ntff_profile_hook = lambda h: state.__setitem__("hook", h)
    mod.get_axon_ntff_profile_hook = lambda: state["hook"]
    sys.modules["antenv.axon_hooks"] = mod
    try:
        import antenv

        antenv.axon_hooks = mod
    except ImportError:
        pass
    try:
        from trn_agent_boot.trn_boot import _ntff_profile_via_ctypes

        mod.set_axon_ntff_profile_hook(
            _ntff_profile_via_ctypes("/opt/axon/libaxon_pjrt.so")
        )
    except Exception:
        pass


def run(inputs, trace=False, **kwargs):
    """Run on the 8 NeuronCores; returns (output, BassKernelResults)."""
    from concourse.bass_utils import run_bass_kernel_spmd

    if trace:
        _ensure_ntff_hook()
    nc = _get_nc()
    in_maps = _prepare_in_maps(inputs)
    res = run_bass_kernel_spmd(
        nc, in_maps, core_ids=list(range(NCORES)), trace=trace, **kwargs
    )
    return _assemble(inputs, res.results), res


def kernel(**inputs) -> np.ndarray:
    out, _ = run(inputs)
    return out


# revision 14
# speedup vs baseline: 1.0093x; 1.0093x over previous
"""COIL sparse-attention scoring kernel for 8 Trainium2 NeuronCores.

Strategy
--------
Shard the doc axis (Bd=128) across the 8 cores (16 docs each); qry tensors are
replicated. The exact-token-match mask is folded INTO the matmul: each token id
(vocab 1000) is encoded as three base-10 digit one-hots scaled by ALPHA=32 and
appended to the reps. Then

    v[qs, ct] = <qry_ext[qs], doc_ext[ct]> = S[qs, ct] + 1024 * match_digits

where match_digits == 3 iff the ids are equal, so

    tok[qs, c] = relu(max_t v[qs, c, t] - 3072)

reproduces the reference masked-max exactly (non-match scores sit below 2100,
matches above 3000). The qry reps are split hi/lo in bf16 (3 cross terms) so
the matmul runs at full bf16 rate with ~fp32 accuracy: K = 3*32 + 30 = 126.

Per core: 32 q-tiles of 128 q-positions; each q-tile is one [126,128]x[126,2048]
matmul into PSUM [128, 2048]. The per-doc max over the 128 doc tokens is split
between the DVE (direct tensor_reduce from PSUM) and a ScalarE relu-convert to
fp16 followed by a DVE tensor_tensor max tree at 2x rate. The sum over query
positions is a ones-vector matmul on the PE (partition-dim sum). CLS scores,
the tiny 4-way q-tile fold, and the final max over the 8 query chunks are done
on host (a few thousand elements).
"""

import os
import numpy as np
import ml_dtypes

Bq, Sq, Bd, Sd, D, Dc = 8, 512, 128, 128, 32, 768
NCORES = 8
BD_PER = Bd // NCORES          # 16 docs per core
K_EXT = 126                    # 32*3 rep dims + 30 one-hot dims
SQF = Bq * Sq                  # 4096 query positions
DCOL = BD_PER * Sd             # 2048 doc tokens per core
NQT = SQF // 128               # 32 q-tiles
ALPHA = 32.0
OFF = 3.0 * ALPHA * ALPHA      # 3072: offset of a full 3-digit match
# q-tile qt goes to the DVE-direct path iff qt % DIRECT_PERIOD == DIRECT_PERIOD-1;
# the rest go ScalarE-relu-fp16 -> DVE max tree. Whole-tile assignment keeps each
# PSUM tile single-reader (fewer semaphore waits).
DIRECT_PERIOD = int(os.environ.get("KERNEL_DIRECT_PERIOD", "5"))
TREE_LEVELS = int(os.environ.get("KERNEL_TREE_LEVELS", "3"))

_CACHE = {}


def _bf16(x):
    return x.astype(ml_dtypes.bfloat16)


def _onehot_digits(ids):
    """ids [N] int in [0,1000) -> [N,30] base-10 digit one-hot (float32)."""
    n = ids.shape[0]
    H = np.zeros((n, 30), dtype=np.float32)
    r = np.arange(n)
    H[r, ids % 10] = 1.0
    H[r, 10 + (ids // 10) % 10] = 1.0
    H[r, 20 + ids // 100] = 1.0
    return H


def _build_qry_ext(qry_reps, qry_input_ids, qry_attention_mask):
    q = np.asarray(qry_reps, np.float32).reshape(SQF, D)
    ids = np.asarray(qry_input_ids, np.int64).reshape(SQF)
    q_hi = _bf16(q).astype(np.float32)
    q_lo = _bf16(q - q_hi).astype(np.float32)
    H = ALPHA * _onehot_digits(ids)
    ext = np.concatenate([q_hi, q_lo, q_hi, H], axis=1)  # [SQF, 126]
    # rows that must contribute 0: CLS (s=0), SEP (last attended pos), mask==0
    mask = np.asarray(qry_attention_mask, np.int64).copy()
    sep = mask.sum(axis=1) - 1
    mask[np.arange(Bq), sep] = 0
    mask[:, 0] = 0
    ext *= mask.reshape(SQF, 1).astype(np.float32)
    return np.ascontiguousarray(_bf16(ext).T)  # [126, SQF]


def _build_doc_ext(doc_reps, doc_input_ids):
    d = np.asarray(doc_reps, np.float32).reshape(-1, D)
    ids = np.asarray(doc_input_ids, np.int64).reshape(-1)
    d_hi = _bf16(d).astype(np.float32)
    d_lo = _bf16(d - d_hi).astype(np.float32)
    H = ALPHA * _onehot_digits(ids)
    ext = np.concatenate([d_hi, d_hi, d_lo, H], axis=1)  # [N, 126]
    return np.ascontiguousarray(_bf16(ext).T)  # [126, N]


def _split_multi_waits(nc, mybir):
    """This container's walrus accepts only ONE sync-wait per instruction
    ("Too many sync wait commands"). Hoist extra waits into standalone
    EventSemaphore instructions on the same engine right before the offender
    (the sequencer blocks on each in order — semantically identical)."""
    n = 0
    for func in nc.m.functions:
        for bb in func.blocks:
            out = []
            for inst in bb.instructions:
                si = inst.sync_info
                if si is not None and len(si.on_wait) > 1:
                    waits = list(si.on_wait)
                    for w in waits[:-1]:
                        n += 1
                        out.append(
                            mybir.InstEventSemaphore(
                                name=f"W-{inst.name}-{n}",
                                engine=inst.engine,
                                ins=[],
                                outs=[],
                                debug=inst.debug,
                                sync_info=mybir.SyncInfo(
                                    on_wait=[w], on_update=[]
                                ),
                            )
                        )
                    inst.sync_info = mybir.SyncInfo(
                        on_wait=[waits[-1]], on_update=list(si.on_update)
                    )
                out.append(inst)
            bb.instructions = out
    return n


def _build_nc(direct_period, tree_levels):
    import concourse.bass as bass
    import concourse.mybir as mybir
    import concourse.tile as tile
    from concourse.bass import ts

    bf16, f16, f32 = mybir.dt.bfloat16, mybir.dt.float16, mybir.dt.float32
    nc = bass.Bass("TRN2", target_bir_lowering=False, debug=False)
    qryT = nc.dram_tensor("qryT", [K_EXT, SQF], bf16, kind="ExternalInput").ap()
    docT = nc.dram_tensor("docT", [K_EXT, DCOL], bf16, kind="ExternalInput").ap()
    out = nc.dram_tensor("out", [1, 16 * NQT], f32, kind="ExternalOutput").ap()

    is_direct = [
        direct_period > 0 and qt % direct_period == direct_period - 1
        for qt in range(NQT)
    ]
    n_direct = sum(is_direct)
    with tile.TileContext(nc) as tc:
        with (
            tc.tile_pool(name="inp", bufs=1) as inp,
            tc.tile_pool(name="psum", bufs=2, space="PSUM") as psum,
            tc.tile_pool(name="stage", bufs=3) as stp,
            tc.tile_pool(name="tree", bufs=2) as trp,
            tc.tile_pool(name="accp", bufs=1) as accp,
        ):
            qry_sb = inp.tile([K_EXT, SQF], bf16)
            for j in range(SQF // 512):
                nc.sync.dma_start(qry_sb[:, ts(j, 512)], qryT[:, ts(j, 512)])
            doc_sb = inp.tile([K_EXT, DCOL], bf16)
            for j in range(DCOL // 512):
                nc.sync.dma_start(doc_sb[:, ts(j, 512)], docT[:, ts(j, 512)])

            accum = accp.tile([128, 16 * NQT], f32)
            nc.vector.memset(accum[:], 0.0)
            draw = accp.tile([128, 16 * max(n_direct, 1)], f32)
            ones = accp.tile([128, 1], f32)
            nc.vector.memset(ones[:], 1.0)

            di = 0
            for qt in range(NQT):
                ps = psum.tile([128, DCOL], f32, tag="score")
                for j in range(DCOL // 512):
                    nc.tensor.matmul(
                        ps[:, ts(j, 512)],
                        qry_sb[:, ts(qt, 128)],
                        doc_sb[:, ts(j, 512)],
                        start=True,
                        stop=True,
                    )
                if is_direct[qt]:
                    # whole tile on DVE straight from PSUM (raw v scale)
                    nc.vector.reduce_max(
                        draw[:, di * 16 : (di + 1) * 16],
                        ps[:].rearrange("p (c t) -> p c t", t=Sd),
                        axis=mybir.AxisListType.X,
                    )
                    di += 1
                else:
                    # fp16 copy of v - OFF on ScalarE; relu applied after the
                    # max tree (max commutes with the shift and the relu)
                    st = stp.tile([128, BD_PER * Sd], f16, tag="stage")
                    nc.scalar.activation(
                        st[:],
                        ps[:],
                        mybir.ActivationFunctionType.Copy,
                        bias=-OFF,
                    )
                    cur, width = st, Sd
                    for lev in range(tree_levels):
                        nxt = trp.tile([128, BD_PER * width // 2], f16, tag=f"t{lev}")
                        cv = cur[:].rearrange("p (c t) -> p c t", t=width)
                        nc.vector.tensor_max(
                            nxt[:].rearrange("p (c t) -> p c t", t=width // 2),
                            cv[:, :, 0 : width // 2],
                            cv[:, :, width // 2 : width],
                        )
                        cur, width = nxt, width // 2
                    nc.vector.reduce_max(
                        accum[:, qt * 16 : (qt + 1) * 16],
                        cur[:].rearrange("p (c t) -> p c t", t=width),
                        axis=mybir.AxisListType.X,
                    )
            if n_direct < NQT:
                # tree q-tiles hold max_t(v) - OFF; apply the relu in place
                # (direct q-tiles' cols are overwritten below)
                nc.vector.tensor_scalar_max(accum[:], accum[:], 0.0)
            di = 0
            for qt in range(NQT):
                if is_direct[qt]:
                    # tok = max(raw, OFF) - OFF into this q-tile's accum cols
                    nc.vector.tensor_scalar(
                        accum[:, qt * 16 : (qt + 1) * 16],
                        draw[:, di * 16 : (di + 1) * 16],
                        OFF,
                        -OFF,
                        mybir.AluOpType.max,
                        mybir.AluOpType.add,
                    )
                    di += 1
            # partition-dim sum over the 128 q-positions of each q-tile
            fin = psum.tile([1, 16 * NQT], f32, tag="score")
            nc.tensor.matmul(fin[:], ones[:], accum[:], start=True, stop=True)
            osb = accp.tile([1, 16 * NQT], f32)
            nc.vector.tensor_copy(osb[:], fin[:])
            nc.sync.dma_start(out[:], osb[:])
    _split_multi_waits(nc, mybir)
    return nc


def _get_nc():
    key = (DIRECT_PERIOD, TREE_LEVELS)
    if key not in _CACHE:
        _CACHE[key] = _build_nc(*key)
    return _CACHE[key]


def _prepare_in_maps(inputs):
    qT = _build_qry_ext(
        inputs["qry_reps"], inputs["qry_input_ids"], inputs["qry_attention_mask"]
    )
    doc_reps = np.asarray(inputs["doc_reps"], np.float32)
    doc_ids = np.asarray(inputs["doc_input_ids"], np.int64)
    in_maps = []
    for core in range(NCORES):
        sl = slice(core * BD_PER, (core + 1) * BD_PER)
        dT = _build_doc_ext(doc_reps[sl], doc_ids[sl])
        in_maps.append({"qryT": qT, "docT": dT})
    return in_maps


def _assemble(inputs, results):
    toks = np.zeros((Bq, Bd), dtype=np.float32)
    for core in range(NCORES):
        part = np.asarray(results[core]["out"], np.float32).reshape(NQT, 16)
        # col-block layout: [q-tile, doc]; 4 q-tiles per query chunk
        toks[:, core * BD_PER : (core + 1) * BD_PER] = part.reshape(Bq, 4, 16).sum(1)
    cls = np.asarray(inputs["qry_cls"], np.float32) @ np.asarray(
        inputs["doc_cls"], np.float32
    ).T
    scores = toks + cls
    return scores.max(axis=0).reshape(-1).astype(np.float32)


def _ensure_ntff_hook():
    """This container's antenv lacks axon_hooks; synthesize the module and
    register the ctypes-based NTFF profile hook so trace=True works."""
    import sys
    import types

    if "antenv.axon_hooks" in sys.modules:
        return
    mod = types.ModuleType("antenv.axon_hooks")
    state = {"hook": None}
    mod.set_axon_ntff_profile_hook = lambda h: state.__setitem__("hook", h)
    mod.get_axon_ntff_profile_hook = lambda: state["hook"]
    sys.modules["antenv.axon_hooks"] = mod
    try:
        import antenv

        antenv.axon_hooks = mod
    except ImportError:
        pass
    try:
        from trn_agent_boot.trn_boot import _ntff_profile_via_ctypes

        mod.set_axon_ntff_profile_hook(
            _ntff_profile_via_ctypes("/opt/axon/libaxon_pjrt.so")
        )
    except Exception:
        pass


def run(inputs, trace=False, **kwargs):
    """Run on the 8 NeuronCores; returns (output, BassKernelResults)."""
    from concourse.bass_utils import run_bass_kernel_spmd

    if trace:
        _ensure_ntff_hook()
    nc = _get_nc()
    in_maps = _prepare_in_maps(inputs)
    res = run_bass_kernel_spmd(
        nc, in_maps, core_ids=list(range(NCORES)), trace=trace, **kwargs
    )
    return _assemble(inputs, res.results), res


def kernel(**inputs) -> np.ndarray:
    out, _ = run(inputs)
    return out


# revision 17
# speedup vs baseline: 1.0892x; 1.0791x over previous
"""COIL sparse-attention scoring kernel for 8 Trainium2 NeuronCores.

Strategy
--------
Shard the doc axis (Bd=128) across the 8 cores (16 docs each); qry tensors are
replicated. The exact-token-match mask is folded INTO the matmul: each token id
(vocab 1000) is encoded as three base-10 digit one-hots scaled by ALPHA=32 and
appended to the reps. Then

    v[qs, ct] = <qry_ext[qs], doc_ext[ct]> = S[qs, ct] + 1024 * match_digits

where match_digits == 3 iff the ids are equal, so

    tok[qs, c] = relu(max_t v[qs, c, t] - 3072)

reproduces the reference masked-max exactly (non-match scores sit below 2100,
matches above 3000). The qry reps are split hi/lo in bf16 (3 cross terms) so
the matmul runs at full bf16 rate with ~fp32 accuracy: K = 3*32 + 30 = 126.

Per core: 32 q-tiles of 128 q-positions; each q-tile is one [126,128]x[126,2048]
matmul into PSUM [128, 2048]. The per-doc max over the 128 doc tokens is split
between the DVE (direct tensor_reduce from PSUM) and a ScalarE relu-convert to
fp16 followed by a DVE tensor_tensor max tree at 2x rate. The sum over query
positions is a ones-vector matmul on the PE (partition-dim sum). CLS scores,
the tiny 4-way q-tile fold, and the final max over the 8 query chunks are done
on host (a few thousand elements).
"""

import os
import numpy as np
import ml_dtypes

Bq, Sq, Bd, Sd, D, Dc = 8, 512, 128, 128, 32, 768
NCORES = 8
BD_PER = Bd // NCORES          # 16 docs per core
K_EXT = 126                    # 32*3 rep dims + 30 one-hot dims
SQF = Bq * Sq                  # 4096 query positions
DCOL = BD_PER * Sd             # 2048 doc tokens per core
NQT = SQF // 128               # 32 q-tiles
ALPHA = 32.0
OFF = 3.0 * ALPHA * ALPHA      # 3072: offset of a full 3-digit match
# q-tile qt goes to the DVE-direct path iff qt % DIRECT_PERIOD == DIRECT_PERIOD-1;
# the rest go ScalarE-relu-fp16 -> DVE max tree. Whole-tile assignment keeps each
# PSUM tile single-reader (fewer semaphore waits).
DIRECT_PERIOD = int(os.environ.get("KERNEL_DIRECT_PERIOD", "5"))
TREE_LEVELS = int(os.environ.get("KERNEL_TREE_LEVELS", "3"))

_CACHE = {}


def _bf16(x):
    return x.astype(ml_dtypes.bfloat16)


def _onehot_digits(ids):
    """ids [N] int in [0,1000) -> [N,30] base-10 digit one-hot (float32)."""
    n = ids.shape[0]
    H = np.zeros((n, 30), dtype=np.float32)
    r = np.arange(n)
    H[r, ids % 10] = 1.0
    H[r, 10 + (ids // 10) % 10] = 1.0
    H[r, 20 + ids // 100] = 1.0
    return H


def _build_qry_ext(qry_reps, qry_input_ids, qry_attention_mask):
    q = np.asarray(qry_reps, np.float32).reshape(SQF, D)
    ids = np.asarray(qry_input_ids, np.int64).reshape(SQF)
    q_hi = _bf16(q).astype(np.float32)
    q_lo = _bf16(q - q_hi).astype(np.float32)
    H = ALPHA * _onehot_digits(ids)
    ext = np.concatenate([q_hi, q_lo, q_hi, H], axis=1)  # [SQF, 126]
    # rows that must contribute 0: CLS (s=0), SEP (last attended pos), mask==0
    mask = np.asarray(qry_attention_mask, np.int64).copy()
    sep = mask.sum(axis=1) - 1
    mask[np.arange(Bq), sep] = 0
    mask[:, 0] = 0
    ext *= mask.reshape(SQF, 1).astype(np.float32)
    return np.ascontiguousarray(_bf16(ext).T)  # [126, SQF]


def _build_doc_ext(doc_reps, doc_input_ids):
    d = np.asarray(doc_reps, np.float32).reshape(-1, D)
    ids = np.asarray(doc_input_ids, np.int64).reshape(-1)
    d_hi = _bf16(d).astype(np.float32)
    d_lo = _bf16(d - d_hi).astype(np.float32)
    H = ALPHA * _onehot_digits(ids)
    ext = np.concatenate([d_hi, d_hi, d_lo, H], axis=1)  # [N, 126]
    return np.ascontiguousarray(_bf16(ext).T)  # [126, N]


_LDW_PATCHED = False


def _patch_ldw_opt():
    """bir_verify_and_optimise hardcodes --enable-ldw-opt=false, which makes
    walrus emit one LDWEIGHTS per matmul even when the stationary operand is
    unchanged (4x redundant here). Append =true (last flag wins)."""
    global _LDW_PATCHED
    if _LDW_PATCHED or os.environ.get("KERNEL_NO_LDW_OPT"):
        return
    import concourse.bass_utils as bu

    orig = bu.get_walrus_args

    def patched(*a, **k):
        return orig(*a, **k) + ["--enable-ldw-opt=true"]

    bu.get_walrus_args = patched
    _LDW_PATCHED = True


def _split_multi_waits(nc, mybir):
    """This container's walrus accepts only ONE sync-wait per instruction
    ("Too many sync wait commands"). Hoist extra waits into standalone
    EventSemaphore instructions on the same engine right before the offender
    (the sequencer blocks on each in order — semantically identical)."""
    n = 0
    for func in nc.m.functions:
        for bb in func.blocks:
            out = []
            for inst in bb.instructions:
                si = inst.sync_info
                if si is not None and len(si.on_wait) > 1:
                    waits = list(si.on_wait)
                    for w in waits[:-1]:
                        n += 1
                        out.append(
                            mybir.InstEventSemaphore(
                                name=f"W-{inst.name}-{n}",
                                engine=inst.engine,
                                ins=[],
                                outs=[],
                                debug=inst.debug,
                                sync_info=mybir.SyncInfo(
                                    on_wait=[w], on_update=[]
                                ),
                            )
                        )
                    inst.sync_info = mybir.SyncInfo(
                        on_wait=[waits[-1]], on_update=list(si.on_update)
                    )
                out.append(inst)
            bb.instructions = out
    return n


def _build_nc(direct_period, tree_levels):
    import concourse.bass as bass
    import concourse.mybir as mybir
    import concourse.tile as tile
    from concourse.bass import ts

    bf16, f16, f32 = mybir.dt.bfloat16, mybir.dt.float16, mybir.dt.float32
    nc = bass.Bass("TRN2", target_bir_lowering=False, debug=False)
    qryT = nc.dram_tensor("qryT", [K_EXT, SQF], bf16, kind="ExternalInput").ap()
    docT = nc.dram_tensor("docT", [K_EXT, DCOL], bf16, kind="ExternalInput").ap()
    out = nc.dram_tensor("out", [1, 16 * NQT], f32, kind="ExternalOutput").ap()

    is_direct = [
        direct_period > 0 and qt % direct_period == direct_period - 1
        for qt in range(NQT)
    ]
    n_direct = sum(is_direct)
    with tile.TileContext(nc) as tc:
        with (
            tc.tile_pool(name="inp", bufs=1) as inp,
            tc.tile_pool(name="psum", bufs=2, space="PSUM") as psum,
            tc.tile_pool(name="stage", bufs=3) as stp,
            tc.tile_pool(name="tree", bufs=2) as trp,
            tc.tile_pool(name="accp", bufs=1) as accp,
        ):
            # PE warm-up: ~3.5us of junk matmuls during the DMA head so the
            # HAM clock-gate reaches 8/8 before the real work starts
            scratch = inp.tile([K_EXT, 512], bf16)
            nc.gpsimd.memset(scratch[:], 0.0)
            wps = psum.tile([128, 512], f32, tag="score")
            for _ in range(8):
                nc.tensor.matmul(
                    wps[:], scratch[:, 0:128], scratch[:], start=True, stop=True
                )

            # qry chunk 0 and the doc chunks first so compute can start early
            qry_sb = inp.tile([K_EXT, SQF], bf16)
            doc_sb = inp.tile([K_EXT, DCOL], bf16)
            nc.sync.dma_start(qry_sb[:, ts(0, 512)], qryT[:, ts(0, 512)])
            for j in range(DCOL // 512):
                nc.sync.dma_start(doc_sb[:, ts(j, 512)], docT[:, ts(j, 512)])
            for j in range(1, SQF // 512):
                nc.sync.dma_start(qry_sb[:, ts(j, 512)], qryT[:, ts(j, 512)])

            accum = accp.tile([128, 16 * NQT], f32)
            nc.vector.memset(accum[:], 0.0)
            draw = accp.tile([128, 16 * max(n_direct, 1)], f32)
            ones = accp.tile([128, 1], f32)
            nc.vector.memset(ones[:], 1.0)

            di = 0
            for qt in range(NQT):
                ps = psum.tile([128, DCOL], f32, tag="score")
                for j in range(DCOL // 512):
                    nc.tensor.matmul(
                        ps[:, ts(j, 512)],
                        qry_sb[:, ts(qt, 128)],
                        doc_sb[:, ts(j, 512)],
                        start=True,
                        stop=True,
                    )
                if is_direct[qt]:
                    # whole tile on DVE straight from PSUM (raw v scale)
                    nc.vector.reduce_max(
                        draw[:, di * 16 : (di + 1) * 16],
                        ps[:].rearrange("p (c t) -> p c t", t=Sd),
                        axis=mybir.AxisListType.X,
                    )
                    di += 1
                else:
                    # fp16 copy of v - OFF on ScalarE; relu applied after the
                    # max tree (max commutes with the shift and the relu)
                    st = stp.tile([128, BD_PER * Sd], f16, tag="stage")
                    nc.scalar.activation(
                        st[:],
                        ps[:],
                        mybir.ActivationFunctionType.Copy,
                        bias=-OFF,
                    )
                    cur, width = st, Sd
                    for lev in range(tree_levels):
                        nxt = trp.tile([128, BD_PER * width // 2], f16, tag=f"t{lev}")
                        cv = cur[:].rearrange("p (c t) -> p c t", t=width)
                        nc.vector.tensor_max(
                            nxt[:].rearrange("p (c t) -> p c t", t=width // 2),
                            cv[:, :, 0 : width // 2],
                            cv[:, :, width // 2 : width],
                        )
                        cur, width = nxt, width // 2
                    nc.vector.reduce_max(
                        accum[:, qt * 16 : (qt + 1) * 16],
                        cur[:].rearrange("p (c t) -> p c t", t=width),
                        axis=mybir.AxisListType.X,
                    )
            if n_direct < NQT:
                # tree q-tiles hold max_t(v) - OFF; apply the relu in place
                # (direct q-tiles' cols are overwritten below)
                nc.vector.tensor_scalar_max(accum[:], accum[:], 0.0)
            di = 0
            for qt in range(NQT):
                if is_direct[qt]:
                    # tok = max(raw, OFF) - OFF into this q-tile's accum cols
                    nc.vector.tensor_scalar(
                        accum[:, qt * 16 : (qt + 1) * 16],
                        draw[:, di * 16 : (di + 1) * 16],
                        OFF,
                        -OFF,
                        mybir.AluOpType.max,
                        mybir.AluOpType.add,
                    )
                    di += 1
            # partition-dim sum over the 128 q-positions of each q-tile
            fin = psum.tile([1, 16 * NQT], f32, tag="score")
            nc.tensor.matmul(fin[:], ones[:], accum[:], start=True, stop=True)
            osb = accp.tile([1, 16 * NQT], f32)
            nc.vector.tensor_copy(osb[:], fin[:])
            nc.sync.dma_start(out[:], osb[:])
    _split_multi_waits(nc, mybir)
    return nc


def _get_nc():
    _patch_ldw_opt()
    key = (DIRECT_PERIOD, TREE_LEVELS)
    if key not in _CACHE:
        _CACHE[key] = _build_nc(*key)
    return _CACHE[key]


def _prepare_in_maps(inputs):
    qT = _build_qry_ext(
        inputs["qry_reps"], inputs["qry_input_ids"], inputs["qry_attention_mask"]
    )
    doc_reps = np.asarray(inputs["doc_reps"], np.float32)
    doc_ids = np.asarray(inputs["doc_input_ids"], np.int64)
    in_maps = []
    for core in range(NCORES):
        sl = slice(core * BD_PER, (core + 1) * BD_PER)
        dT = _build_doc_ext(doc_reps[sl], doc_ids[sl])
        in_maps.append({"qryT": qT, "docT": dT})
    return in_maps


def _assemble(inputs, results):
    toks = np.zeros((Bq, Bd), dtype=np.float32)
    for core in range(NCORES):
        part = np.asarray(results[core]["out"], np.float32).reshape(NQT, 16)
        # col-block layout: [q-tile, doc]; 4 q-tiles per query chunk
        toks[:, core * BD_PER : (core + 1) * BD_PER] = part.reshape(Bq, 4, 16).sum(1)
    cls = np.asarray(inputs["qry_cls"], np.float32) @ np.asarray(
        inputs["doc_cls"], np.float32
    ).T
    scores = toks + cls
    return scores.max(axis=0).reshape(-1).astype(np.float32)


def _ensure_ntff_hook():
    """This container's antenv lacks axon_hooks; synthesize the module and
    register the ctypes-based NTFF profile hook so trace=True works."""
    import sys
    import types

    if "antenv.axon_hooks" in sys.modules:
        return
    mod = types.ModuleType("antenv.axon_hooks")
    state = {"hook": None}
    mod.set_axon_ntff_profile_hook = lambda h: state.__setitem__("hook", h)
    mod.get_axon_ntff_profile_hook = lambda: state["hook"]
    sys.modules["antenv.axon_hooks"] = mod
    try:
        import antenv

        antenv.axon_hooks = mod
    except ImportError:
        pass
    try:
        from trn_agent_boot.trn_boot import _ntff_profile_via_ctypes

        mod.set_axon_ntff_profile_hook(
            _ntff_profile_via_ctypes("/opt/axon/libaxon_pjrt.so")
        )
    except Exception:
        pass


def run(inputs, trace=False, **kwargs):
    """Run on the 8 NeuronCores; returns (output, BassKernelResults)."""
    from concourse.bass_utils import run_bass_kernel_spmd

    if trace:
        _ensure_ntff_hook()
    nc = _get_nc()
    in_maps = _prepare_in_maps(inputs)
    res = run_bass_kernel_spmd(
        nc, in_maps, core_ids=list(range(NCORES)), trace=trace, **kwargs
    )
    return _assemble(inputs, res.results), res


def kernel(**inputs) -> np.ndarray:
    out, _ = run(inputs)
    return out
